# revision 9
# baseline (speedup 1.0000x reference)
"""Multi-headed attention Trainium2 kernel.

Problem: B=4, S=2048, D=1024, H=16, dk=dv=64, fp32.
  q = einsum("bsd,hdk->bhsk", x, W_Q); k,v similar
  scores = q@k.T/8; attn = softmax(scores); out = attn@v
  y = concat_heads(out) @ W_O

Sharding: 8 cores = 4-way data parallel (batch) x 2-way tensor parallel
(head groups of 8). Core c handles batch c%4, heads 8*(c//4)..+8. Each core
returns a partial y for its batch; host sums the two head-group partials.

Per-core kernel (S=2048, D=1024, HL=8 local heads):
  phase A: transpose x into xT (PE transpose via identity), project
    K^T (pair-packed [k0|k1] -> [128, S] tiles, resident), Q^T (same, spilled
    to DRAM, reloaded per s-chunk in phase B), V (natural [t, v] layout with
    a ones column per head for the softmax denominator).
  phase B: per (pair, s-chunk): scores^T [t, s] via row-packed K=64 matmuls
    (two heads concurrent in PE row groups), exp on ACT straight from PSUM
    with fused 1/8 scale (no max subtraction: |scores| < ~12 so fp32 exp is
    safe and matches softmax exactly), AV matmul with ones column giving
    sum-exp in row 64, normalize via reciprocal + partition_broadcast + mul,
    then the W_O matmul accumulated over pairs.

All matmuls run in float32r (full PE rate at moving dim >= 256; ~2e-4 rel
err vs fp32). fp32r operands must come from rounding producers (DVE/ACT
casts) or f32r-typed DRAM.
"""

import numpy as np

import concourse.bacc as bacc
import concourse.bass as bass
import concourse.mybir as mybir
import concourse.tile as tile

F32 = mybir.dt.float32
F32R = mybir.dt.float32r
BF16 = mybir.dt.bfloat16
EXP_DT = F32R  # dtype of exp(scores) tiles and V_ones (F32R or BF16)
P = 128
DK = 64  # per-head dim; also dv
TG = 2  # t-tiles per exp group (psum scores tile = [128, TG*2*512])


def build_nc(S, D, HL, num_devices=8, n_iters=1, cfg=None):
    """Build the per-core Bass program. S seq len, D model dim, HL local heads."""
    NSC = S // 512  # s-chunks
    NT = S // P  # t-tiles
    ND = D // P  # d-tiles
    NPAIR = HL // 2
    NDC = max(1, D // 512)  # output d chunks
    DC = min(D, 512)
    VW = DK + 1  # 65: v columns + ones column
    scale = 1.0 / np.sqrt(np.float64(DK))
    cfg = dict(cfg or {})
    tg = cfg.get("tg", TG)
    psS_bufs = cfg.get("psS_bufs", 1)
    psA_bufs = cfg.get("psA_bufs", 2)
    psO_bufs = cfg.get("psO_bufs", 2)
    exp_bufs = cfg.get("exp_bufs", 2)
    ot_bufs = cfg.get("ot_bufs", 5)
    xt_bufs = cfg.get("xt_bufs", ND + 2)
    assert psS_bufs * 2 * tg + psA_bufs + psO_bufs <= 8, "psum overflow"

    nc = bacc.Bacc("TRN2", target_bir_lowering=False, debug=False,
                   num_devices=num_devices)

    x = nc.dram_tensor("x", [S, D], F32, kind="ExternalInput").ap()
    wq = nc.dram_tensor("wq", [HL, D, DK], F32R, kind="ExternalInput").ap()
    wk = nc.dram_tensor("wk", [HL, D, DK], F32R, kind="ExternalInput").ap()
    wv = nc.dram_tensor("wv", [HL, D, DK], F32R, kind="ExternalInput").ap()
    wo = nc.dram_tensor("wo", [HL * DK, D], F32R, kind="ExternalInput").ap()
    ident = nc.dram_tensor("ident", [P, P], F32, kind="ExternalInput").ap()
    ones_in = nc.dram_tensor("ones", [P, NT * HL, 1], F32R, kind="ExternalInput").ap()
    y = nc.dram_tensor("y", [S, D], F32, kind="ExternalOutput").ap()

    qt_spill = nc.dram_tensor("qt_spill", [NPAIR, P, S], F32R).ap()

    from contextlib import ExitStack

    with tile.TileContext(nc) as tc:
        with ExitStack() as ctx:
            pool = lambda name, bufs, **kw: ctx.enter_context(
                tc.tile_pool(name=name, bufs=bufs, **kw)
            )
            persist = pool("persist", 1)
            xload_p = pool("xload", 3)
            xt_p = pool("xt", xt_bufs)
            wqk_p = pool("wqk", 2)
            wo_p = pool("wob", 2)
            qts_p = pool("qts", 3)
            qtb_p = pool("qtb", 3)
            exp_p = pool("exp", exp_bufs)
            ot_p = pool("ot", ot_bufs)
            y_p = pool("ysb", 3)
            rl_p = pool("rl", 3)
            rb_p = pool("rb", 3)
            tmp_p = pool("tmp", 2)
            rl0_p = pool("rl0", 3)
            psA = pool("psA", psA_bufs, space="PSUM")
            psS = pool("psS", psS_bufs, space="PSUM")
            psO = pool("psO", psO_bufs, space="PSUM")
            if n_iters > 1:
                ctx.enter_context(tc.For_i(0, n_iters, 1))
            # --- constants / persistent tiles ---
            id_t = persist.tile([P, P], F32, tag="ident")
            nc.sync.dma_start(id_t[:], ident[:])

            kt = [persist.tile([P, S], F32R, tag=f"kt{p}", name=f"kt{p}") for p in range(NPAIR)]
            v_ones = persist.tile([P, NT * HL * VW], EXP_DT, tag="vones")
            # ones columns (col DK of each per-head block of VW)
            ones_view = v_ones[:].rearrange(
                "p (t h c) -> p (t h) c", h=HL, c=VW
            )[:, :, DK : DK + 1]
            nc.sync.dma_start(ones_view, ones_in[:])

            wv_sb = persist.tile([P, ND * HL * DK], F32R, tag="wv")
            # wv[hl] is [D, DK] -> [128, (dt, hl, k)] with dt-major blocks
            for hl in range(HL):
                nc.sync.dma_start(
                    wv_sb[:].rearrange("p (a h k) -> p a h k", a=ND, h=HL)[
                        :, :, hl, :
                    ],
                    wv[hl].rearrange("(a p) k -> p a k", p=P),
                )

            # --- phase A: transpose x, project K (resident) + Q (spilled) + V ---
            for sc in range(NSC):
                # x^T chunk: ND tiles [128, 512], f32r
                xtc = [xt_p.tile([P, 512], F32R, tag="xt", name=f"xt_{sc}_{dt}") for dt in range(ND)]
                for st in range(4):
                    xl = xload_p.tile([P, D], F32, tag="xl")
                    nc.sync.dma_start(xl[:], x[(sc * 4 + st) * P : (sc * 4 + st + 1) * P, :])
                    for dt in range(ND):
                        pst = psA.tile([P, 512], F32, tag="ps")
                        nc.tensor.transpose(
                            pst[:, :P], xl[:, dt * P : (dt + 1) * P], id_t[:]
                        )
                        nc.vector.tensor_copy(
                            xtc[dt][:, st * P : (st + 1) * P], pst[:, :P]
                        )

                # Q/K projections, pair-packed: lhsT = [w(2p) | w(2p+1)] [128d, 128]
                for p in range(NPAIR):
                    for w_dram, is_q in ((wq, True), (wk, False)):
                        wt = wqk_p.tile([P, ND * P], F32R, tag="wqk")
                        for j in range(2):
                            nc.sync.dma_start(
                                wt[:].rearrange("p (a j k) -> p a j k", a=ND, j=2)[
                                    :, :, j, :
                                ],
                                w_dram[2 * p + j].rearrange("(a p) k -> p a k", p=P),
                            )
                        ps = psA.tile([P, 512], F32, tag="ps")
                        for dt in range(ND):
                            nc.tensor.matmul(
                                ps[:],
                                wt[:, dt * P : (dt + 1) * P],
                                xtc[dt][:],
                                start=(dt == 0),
                                stop=(dt == ND - 1),
                            )
                        if is_q:
                            qs = qts_p.tile([P, 512], F32R, tag="qts")
                            nc.vector.tensor_copy(qs[:], ps[:])
                            nc.sync.dma_start(
                                qt_spill[p, :, sc * 512 : (sc + 1) * 512], qs[:]
                            )
                        else:
                            nc.vector.tensor_copy(
                                kt[p][:, sc * 512 : (sc + 1) * 512], ps[:]
                            )

                # V projection: natural [t, v] layout, all HL heads in one psum
                for st in range(4):
                    tt = sc * 4 + st
                    ps = psA.tile([P, 512], F32, tag="ps")
                    for dt in range(ND):
                        nc.tensor.matmul(
                            ps[:, : HL * DK],
                            xtc[dt][:, st * P : (st + 1) * P],
                            wv_sb[:, dt * HL * DK : (dt + 1) * HL * DK],
                            start=(dt == 0),
                            stop=(dt == ND - 1),
                        )
                    nc.vector.tensor_copy(
                        v_ones[:].rearrange("p (t h c) -> p t h c", h=HL, c=VW)[
                            :, tt, :, :DK
                        ],
                        ps[:, : HL * DK].rearrange("p (h k) -> p h k", h=HL),
                    )

            # --- phase B: attention per (pair, s-chunk) + output projection ---
            NG = NT // tg  # exp groups per (p, sc)
            GW = tg * 512  # free width per head in the scores psum tile
            for sc in range(NSC):
                ots = []
                for p in range(NPAIR):
                    qtb = qtb_p.tile([P, 512], F32R, tag="qtb")
                    nc.sync.dma_start(qtb[:], qt_spill[p, :, sc * 512 : (sc + 1) * 512])

                    po_e = psO.tile([P, 512], F32, tag="av")
                    po_o = psO.tile([P, 512], F32, tag="av")
                    for g in range(NG):
                        pse = psS.tile([P, 2 * GW], F32, tag="sc")
                        for j in range(tg):
                            tt = g * tg + j
                            for h in range(2):  # row-packed head pair
                                nc.tensor.matmul(
                                    pse[:, h * GW + j * 512 : h * GW + (j + 1) * 512],
                                    kt[p][h * DK : (h + 1) * DK, tt * P : (tt + 1) * P],
                                    qtb[h * DK : (h + 1) * DK, :],
                                    start=True,
                                    stop=True,
                                )
                        et = exp_p.tile([P, 2 * GW], EXP_DT, tag="exp")
                        nc.scalar.activation(
                            et[:], pse[:], mybir.ActivationFunctionType.Exp,
                            scale=float(scale),
                        )
                        for j in range(tg):
                            tt = g * tg + j
                            for h, po in ((0, po_e), (1, po_o)):
                                nc.tensor.matmul(
                                    po[:VW, :],
                                    v_ones[
                                        :,
                                        (tt * HL + 2 * p + h) * VW : (tt * HL + 2 * p + h + 1) * VW,
                                    ],
                                    et[:, h * GW + j * 512 : h * GW + (j + 1) * 512],
                                    start=(g == 0 and j == 0),
                                    stop=(g == NG - 1 and j == tg - 1),
                                )

                    # normalize: rows 0:64 divided by row 64 (sum of exp)
                    ot = ot_p.tile([P, 512], F32R, tag="ot")
                    ots.append(ot)
                    for h, po in ((0, po_e), (1, po_o)):
                        rl = rl_p.tile([VW, 512], F32, tag="rl")
                        nc.vector.reciprocal(rl[DK : DK + 1, :], po[DK : DK + 1, :])
                        # partition_broadcast reads physical partition 0 on HW:
                        # hop the row down first
                        rl0 = rl0_p.tile([1, 512], F32, tag="rl0")
                        nc.sync.dma_start(rl0[:], rl[DK : DK + 1, :])
                        rb = rb_p.tile([DK, 512], F32, tag="rb")
                        nc.gpsimd.partition_broadcast(rb[:], rl0[:], channels=DK)
                        if h == 0:
                            nc.vector.tensor_mul(ot[:DK, :], po[:DK, :], rb[:])
                        else:
                            tmp = tmp_p.tile([DK, 512], F32R, tag="tmp")
                            nc.vector.tensor_mul(tmp[:], po[:DK, :], rb[:])
                            nc.sync.dma_start(ot[DK:P, :], tmp[:])

                # output projection for this s-chunk
                for dc in range(NDC):
                    wos = wo_p.tile([P, NPAIR * DC], F32R, tag="wo")
                    for p in range(NPAIR):
                        nc.sync.dma_start(
                            wos[:, p * DC : (p + 1) * DC],
                            wo.rearrange("(a p) d -> a p d", p=P)[
                                p, :, dc * DC : (dc + 1) * DC
                            ],
                        )
                    for st in range(4):
                        psy = psA.tile([P, 512], F32, tag="ps")
                        for p in range(NPAIR):
                            nc.tensor.matmul(
                                psy[:, :DC],
                                ots[p][:, st * P : (st + 1) * P],
                                wos[:, p * DC : (p + 1) * DC],
                                start=(p == 0),
                                stop=(p == NPAIR - 1),
                            )
                        ys = y_p.tile([P, DC], F32, tag="ysb")
                        nc.vector.tensor_copy(ys[:], psy[:, :DC])
                        nc.sync.dma_start(
                            y[(sc * 4 + st) * P : (sc * 4 + st + 1) * P,
                              dc * DC : (dc + 1) * DC],
                            ys[:],
                        )

    nc.compile()
    return nc


_NC_CACHE = {}


def _get_nc(S, D, HL):
    key = (S, D, HL)
    if key not in _NC_CACHE:
        _NC_CACHE[key] = build_nc(S, D, HL)
    return _NC_CACHE[key]


def make_in_maps(x, W_Q, W_K, W_V, W_O, n_cores=8):
    """Shard full inputs into per-core in_maps (DP over batch x TP over heads)."""
    B = x.shape[0]
    H = W_Q.shape[0]
    n_groups = n_cores // B
    HL = H // n_groups
    ident = np.eye(P, dtype=np.float32)
    S = x.shape[1]
    ones = np.ones((P, (S // P) * HL, 1), dtype=np.float32)
    in_maps = []
    for c in range(n_cores):
        b, g = c % B, c // B
        hs = slice(g * HL, (g + 1) * HL)
        in_maps.append({
            "x": np.ascontiguousarray(x[b]),
            "wq": np.ascontiguousarray(W_Q[hs]),
            "wk": np.ascontiguousarray(W_K[hs]),
            "wv": np.ascontiguousarray(W_V[hs]),
            "wo": np.ascontiguousarray(W_O[g * HL * DK : (g + 1) * HL * DK]),
            "ident": ident,
            "ones": ones,
        })
    return in_maps


def kernel(x, W_Q, W_K, W_V, W_O):
    from concourse.bass_utils import run_bass_kernel_spmd

    B, S, D = x.shape
    H = W_Q.shape[0]
    n_cores = 8
    HL = H // (n_cores // B)
    nc = _get_nc(S, D, HL)
    in_maps = make_in_maps(x, W_Q, W_K, W_V, W_O, n_cores)
    res = run_bass_kernel_spmd(nc, in_maps, core_ids=list(range(n_cores)))
    y = np.empty((B, S, D), dtype=np.float32)
    for b in range(B):
        y[b] = res.results[b]["y"]
        for g in range(1, n_cores // B):
            y[b] += res.results[g * B + b]["y"]
    return y
